# revision 13
# baseline (speedup 1.0000x reference)
"""Trainium2 Bass kernel for nn_CrossAttention (sparse_attention).

Sharding: data-parallel over B across 8 NeuronCores (1 batch element per
core, weights replicated, no collectives).

Per-core algorithm (T=4096, N=512, S=512, D=1024, H=16, dh=64):
  - exact restructuring: y_tot[t,h,:] = q_sm[t,h,:] @ attn2[h]  where
    attn2[h] = attn[h] + ones(dh) x sattnsum[h]   (exact because
    sum_d q_sm[t,h,d] * sattnsum[h,l] == qsum*sattnsum == reference sy)
  - softmax normalizations deferred: attn[h] = diag(1/colsum(Ek)_h) @ Ek_h^T Vh
    sattnsum[h,l] = sum_d recip_s[h,d] * (Es_h^T Vs_h)[d,l]
  - no softmax max-subtraction (values are small; exp is safe)
  - activations kept token-major in SBUF; PE-transposes feed the matmuls
  - TensorE compute in bf16 (validated fro-rel ~2.3e-3 vs f32 reference)
"""
import numpy as np

H, D, TFD, AUD, EPS = 16, 1024, 256, 768, 1e-5
B, T, N, S = 8, 4096, 512, 512
dh = D // H
P = 128
TT = T // P           # 32 token tiles
NT = 2 * N // P       # 8 n tiles
ST = S // P           # 4 s tiles
DC = D // P           # 8 feature chunks
NCORES = 8

_CACHE = {}


def _build(affine_x, affine_t, affine_s, dbg=False):
    import concourse.bass as bass
    import concourse.tile as tile
    from concourse import bacc, mybir
    from concourse.masks import make_identity

    FP32 = mybir.dt.float32
    BF16 = mybir.dt.bfloat16
    AX = mybir.AxisListType.X
    AF = mybir.ActivationFunctionType
    OP = mybir.AluOpType

    nc = bacc.Bacc()

    # ---------------- DRAM parameters (per-core shapes) ----------------
    x_ext = nc.declare_dram_parameter("x", [T, D], FP32, isOutput=False)
    xf_ext = nc.declare_dram_parameter("xf", [AUD], FP32, isOutput=False)
    xw_ext = nc.declare_dram_parameter("xw", [N, TFD], FP32, isOutput=False)
    xs_ext = nc.declare_dram_parameter("xs", [S, D], FP32, isOutput=False)
    wext = {}
    for nm, shp in [
        ("norm_g", [D]), ("norm_b", [D]), ("tnorm_g", [D]), ("tnorm_b", [D]),
        ("snorm_g", [D]), ("snorm_b", [D]),
        ("Wq", [D, D]), ("bq", [D]), ("Wk", [D, D]), ("bk", [D]),
        ("Wv", [D, D]), ("bv", [D]), ("Wa", [AUD, TFD]), ("ba", [TFD]),
        ("Wat", [TFD, D]), ("bat", [D]), ("Wo", [D, D]), ("bo", [D]),
    ]:
        wext[nm] = nc.declare_dram_parameter(nm, shp, FP32, isOutput=False)
    out_ext = nc.declare_dram_parameter("out", [T, D], FP32, isOutput=True)
    dext = {}
    if dbg:
        for nm, shp in [
            ("d_xfpT", [P, 2]), ("d_xcT", [P, 2, 2 * N]),
            ("d_ek", [NT, P, D]), ("d_v", [NT, P, D]),
            ("d_recipk", [P, DC]), ("d_recips", [P, DC]),
            ("d_attnsc", [P, DC, P]), ("d_sattn", [1, DC, P]),
            ("d_attn2", [P, DC, P]), ("d_eq", [TT, P, D]),
            ("d_qsm", [TT, P, D]), ("d_siluT", [TT, P, DC, P]),
        ]:
            dext[nm] = nc.declare_dram_parameter(nm, shp, FP32, isOutput=True)

    with tile.TileContext(nc) as tc, \
         tc.tile_pool(name="wpool", bufs=1) as wpool, \
         tc.tile_pool(name="npool", bufs=1) as npool, \
         tc.tile_pool(name="work", bufs=2) as work, \
         tc.tile_pool(name="xin", bufs=3) as xin, \
         tc.tile_pool(name="pproj", bufs=2, space="PSUM") as pproj, \
         tc.tile_pool(name="pacc", bufs=2, space="PSUM") as pacc, \
         tc.tile_pool(name="ptp", bufs=2, space="PSUM") as ptp, \
         tc.tile_pool(name="psmall", bufs=2, space="PSUM") as psmall:

        # ---------------- constants ----------------
        ident_bf = wpool.tile([P, P], BF16, tag="ident_bf")
        make_identity(nc, ident_bf)
        ones1_bf = wpool.tile([1, P], BF16, tag="ones1_bf")
        nc.vector.memset(ones1_bf, 1.0)
        ones1_f = wpool.tile([1, P], FP32, tag="ones1_f")
        nc.vector.memset(ones1_f, 1.0)
        onescol_bf = wpool.tile([P, 1], BF16, tag="onescol_bf")
        nc.vector.memset(onescol_bf, 1.0)
        onescol_f = wpool.tile([P, 1], FP32, tag="onescol_f")
        nc.vector.memset(onescol_f, 1.0)
        mask_f = wpool.tile([P, P], FP32, tag="mask_f")
        nc.vector.memset(mask_f, 0.0)
        nc.vector.memset(mask_f[0:dh, 0:dh], 1.0)
        nc.vector.memset(mask_f[dh:P, dh:P], 1.0)
        eps_t = wpool.tile([P, 1], FP32, tag="eps_t")
        nc.vector.memset(eps_t, EPS)

        # ---------------- weights (DMA-cast f32 -> bf16) ----------------
        def load_w(nm, rows, cols):
            t = wpool.tile([P, rows // P, cols], BF16, tag=nm)
            nc.gpsimd.dma_start(
                out=t, in_=wext[nm][:, :].rearrange("(c p) n -> p c n", p=P))
            return t

        Wq_sb = load_w("Wq", D, D)
        Wk_sb = load_w("Wk", D, D)
        Wv_sb = load_w("Wv", D, D)
        Wo_sb = load_w("Wo", D, D)
        Wat_sb = load_w("Wat", TFD, D)
        Wa_sb = load_w("Wa", AUD, TFD)

        def load_row(nm, L):
            t = wpool.tile([1, L], BF16, tag=nm + "_r")
            nc.gpsimd.dma_start(out=t, in_=wext[nm][:][None, :])
            return t

        bq_r = load_row("bq", D)
        bk_r = load_row("bk", D)
        bv_r = load_row("bv", D)
        bat_r = load_row("bat", D)
        bo_r = load_row("bo", D)
        ba_r = load_row("ba", TFD)
        xf_col = wpool.tile([P, AUD // P], BF16, tag="xf_col")
        nc.gpsimd.dma_start(out=xf_col,
                            in_=xf_ext[:].rearrange("(c p) -> p c", p=P))

        def bcast_vec(nm):
            # [D] dram vector -> [P, D] sbuf (same row on all partitions)
            t = wpool.tile([P, D], FP32, tag=nm + "_bc")
            src = wext[nm][:][None, :].broadcast_to([P, D])
            nc.gpsimd.dma_start(out=t, in_=src)
            return t

        gx_bc = bcast_vec("norm_g") if affine_x else None
        bx_bc = bcast_vec("norm_b") if affine_x else None
        gt_bc = bcast_vec("tnorm_g") if affine_t else None
        bt_bc = bcast_vec("tnorm_b") if affine_t else None
        gs_bc = bcast_vec("snorm_g") if affine_s else None
        bs_bc = bcast_vec("snorm_b") if affine_s else None

        # ---------------- n1: xf_projT [P, 2] f32 ----------------
        xfpT = npool.tile([P, 2], FP32, tag="xfpT")
        for m in range(2):
            ps = psmall.tile([P, 1], FP32, tag="small")
            for ac in range(AUD // P):
                nc.tensor.matmul(ps, lhsT=Wa_sb[:, ac, m * P:(m + 1) * P],
                                 rhs=xf_col[:, ac:ac + 1],
                                 start=(ac == 0), stop=False)
            # ba folded in via a K=1 matmul: ps += ba_chunk^T @ [1]
            nc.tensor.matmul(ps, lhsT=ba_r[0:1, m * P:(m + 1) * P],
                             rhs=ones1_bf[0:1, 0:1], start=False, stop=True)
            nc.vector.tensor_copy(out=xfpT[:, m:m + 1], in_=ps)
        if dbg:
            nc.gpsimd.dma_start(out=dext["d_xfpT"][:, :], in_=xfpT)

        # ---------------- n2: xcT [P, 2, 2N] bf16 ----------------
        xcT = npool.tile([P, 2, 2 * N], BF16, tag="xcT")
        for nt in range(N // P):
            xw_t = work.tile([P, TFD], BF16, tag="xw_t")
            nc.gpsimd.dma_start(out=xw_t, in_=xw_ext[nt * P:(nt + 1) * P, :])
            for tc2 in range(2):
                tp = ptp.tile([P, P], BF16, tag="tpbf")
                nc.tensor.transpose(tp, xw_t[:, tc2 * P:(tc2 + 1) * P], ident_bf)
                nc.vector.tensor_copy(out=xcT[:, tc2, nt * P:(nt + 1) * P], in_=tp)
        for tc2 in range(2):
            nc.vector.tensor_copy(out=xcT[:, tc2, N:2 * N],
                                  in_=xfpT[:, tc2:tc2 + 1].broadcast_to([P, N]))
        if dbg:
            nc.gpsimd.dma_start(out=dext["d_xcT"][:, :, :], in_=xcT)

        # ---------------- shared LN helper (token-major) ----------------
        def ln_from_psum(ps_halves, out_bf, g_bc, b_bc):
            """LN over free dim D given 2 psum tiles [P,512]; out bf16 [P,D]."""
            stats = work.tile([P, 2, 6], FP32, tag="stats")
            for j, ps in enumerate(ps_halves):
                nc.vector.bn_stats(out=stats[:, j, :], in_=ps)
            mv = work.tile([P, 2], FP32, tag="mv")
            nc.vector.bn_aggr(out=mv, in_=stats)
            nc.scalar.activation(out=mv[:, 1:2], in_=mv[:, 1:2],
                                 func=AF.Sqrt, bias=eps_t, scale=1.0)
            nc.vector.reciprocal(out=mv[:, 1:2], in_=mv[:, 1:2])
            for j, ps in enumerate(ps_halves):
                dst = out_bf[:, j * 512:(j + 1) * 512]
                if g_bc is None:
                    nc.vector.tensor_scalar(
                        out=dst, in0=ps, scalar1=mv[:, 0:1], scalar2=mv[:, 1:2],
                        op0=OP.subtract, op1=OP.mult)
                else:
                    tmpf = work.tile([P, 512], FP32, tag="lnt")
                    nc.vector.tensor_scalar(
                        out=tmpf, in0=ps, scalar1=mv[:, 0:1], scalar2=mv[:, 1:2],
                        op0=OP.subtract, op1=OP.mult)
                    nc.vector.tensor_mul(out=tmpf, in0=tmpf,
                                         in1=g_bc[:, j * 512:(j + 1) * 512])
                    nc.vector.tensor_add(out=dst, in0=tmpf,
                                         in1=b_bc[:, j * 512:(j + 1) * 512])

        def ln_from_sbuf(x_f32, out_bf, g_bc, b_bc):
            """LN over free dim D given sbuf f32 [P, D]; out bf16 [P, D]."""
            stats = work.tile([P, 2, 6], FP32, tag="stats")
            for j in range(2):
                nc.vector.bn_stats(out=stats[:, j, :],
                                   in_=x_f32[:, j * 512:(j + 1) * 512])
            mv = work.tile([P, 2], FP32, tag="mv")
            nc.vector.bn_aggr(out=mv, in_=stats)
            nc.scalar.activation(out=mv[:, 1:2], in_=mv[:, 1:2],
                                 func=AF.Sqrt, bias=eps_t, scale=1.0)
            nc.vector.reciprocal(out=mv[:, 1:2], in_=mv[:, 1:2])
            if g_bc is None:
                nc.vector.tensor_scalar(
                    out=out_bf, in0=x_f32, scalar1=mv[:, 0:1], scalar2=mv[:, 1:2],
                    op0=OP.subtract, op1=OP.mult)
            else:
                tmpf = work.tile([P, D], FP32, tag="lnt2")
                nc.vector.tensor_scalar(
                    out=tmpf, in0=x_f32, scalar1=mv[:, 0:1], scalar2=mv[:, 1:2],
                    op0=OP.subtract, op1=OP.mult)
                nc.vector.tensor_mul(out=tmpf, in0=tmpf, in1=g_bc)
                nc.vector.tensor_add(out=out_bf, in0=tmpf, in1=b_bc)

        def transpose_to(src_bf, dstT):
            """src_bf [P, nchunk*P] token-major bf16 -> dstT [P, nchunk, P]."""
            nchunk = src_bf.shape[-1] // P
            for g in range(0, nchunk, 4):
                cnt = min(4, nchunk - g)
                tps = ptp.tile([P, 512], BF16, tag="tpbf")
                for k in range(cnt):
                    c = g + k
                    nc.tensor.transpose(tps[:, k * P:(k + 1) * P],
                                        src_bf[:, c * P:(c + 1) * P], ident_bf)
                nc.vector.tensor_copy(
                    out=dstT[:, g:g + cnt, :],
                    in_=tps[:, 0:cnt * P].rearrange("p (a b) -> p a b", a=cnt))

        def proj_to_psum(actT, w_sb, b_r, jh):
            """psum [P,512] = actT.T @ W[:, jh half] + bias broadcast."""
            ps = pproj.tile([P, 512], FP32, tag="proj")
            for dc in range(DC):
                nc.tensor.matmul(ps, lhsT=actT[:, dc, :],
                                 rhs=w_sb[:, dc, jh * 512:(jh + 1) * 512],
                                 start=(dc == 0), stop=False)
            nc.tensor.matmul(ps, lhsT=ones1_bf,
                             rhs=b_r[0:1, jh * 512:(jh + 1) * 512],
                             start=False, stop=True)
            return ps

        # ---------------- n3 + s-path: K/V + attn accumulation ----------
        # attn/G accumulators: blocks c=0..3 in [:, c*128...] of acc0,
        # c=4..7 in acc1 (packed, held across the sequence loop).
        def kv_attn_phase(nseq_tiles, recip_dst, make_actT, dump=False):
            acc0 = pacc.tile([P, 512], FP32, tag="acc")
            acc1 = pacc.tile([P, 512], FP32, tag="acc")
            acc = [acc0, acc1]
            dT = psmall.tile([P, DC], FP32, tag="small")
            # Interleaved accumulation chains share banks; a start=True
            # would clear the whole bank's has_written bits and drop the
            # sibling chains' partials. Zero the banks once, then
            # accumulate with start=False (overwrite-where-clear).
            nc.vector.memset(acc0, 0.0)
            nc.vector.memset(acc1, 0.0)
            nc.vector.memset(dT, 0.0)
            for it in range(nseq_tiles):
                actT = make_actT(it)
                ek = work.tile([P, D], BF16, tag="ek_t")
                vv = work.tile([P, D], BF16, tag="v_t")
                for w_sb, b_r, is_k in ((Wk_sb, bk_r, True), (Wv_sb, bv_r, False)):
                    for jh in range(2):
                        ps = proj_to_psum(actT, w_sb, b_r, jh)
                        if is_k:
                            nc.scalar.activation(
                                out=ek[:, jh * 512:(jh + 1) * 512],
                                in_=ps, func=AF.Exp)
                        else:
                            nc.vector.tensor_copy(
                                out=vv[:, jh * 512:(jh + 1) * 512], in_=ps)
                if dbg and dump:
                    nc.gpsimd.dma_start(out=dext["d_ek"][it], in_=ek)
                    nc.gpsimd.dma_start(out=dext["d_v"][it], in_=vv)
                last = it == nseq_tiles - 1
                for c in range(DC):
                    nc.tensor.matmul(acc[c // 4][:, (c % 4) * P:(c % 4 + 1) * P],
                                     lhsT=ek[:, c * P:(c + 1) * P],
                                     rhs=vv[:, c * P:(c + 1) * P],
                                     start=False, stop=last,
                                     skip_group_check=True)
                for dc in range(DC):
                    nc.tensor.matmul(dT[:, dc:dc + 1],
                                     lhsT=ek[:, dc * P:(dc + 1) * P],
                                     rhs=onescol_bf,
                                     start=False, stop=last,
                                     skip_group_check=True)
            nc.vector.reciprocal(out=recip_dst, in_=dT)
            return acc

        # --- n-path (2N = 1024 rows: xw_xf -> tn) ---
        def make_tnT(nt):
            psa = pproj.tile([P, 512], FP32, tag="proj")
            psb = pproj.tile([P, 512], FP32, tag="proj")
            for jh, ps in enumerate((psa, psb)):
                for tc2 in range(2):
                    nc.tensor.matmul(ps, lhsT=xcT[:, tc2, nt * P:(nt + 1) * P],
                                     rhs=Wat_sb[:, tc2, jh * 512:(jh + 1) * 512],
                                     start=(tc2 == 0), stop=False)
                nc.tensor.matmul(ps, lhsT=ones1_bf,
                                 rhs=bat_r[0:1, jh * 512:(jh + 1) * 512],
                                 start=False, stop=True)
            tn_t = work.tile([P, D], BF16, tag="tn_t")
            ln_from_psum((psa, psb), tn_t, gt_bc, bt_bc)
            tnT = work.tile([P, DC, P], BF16, tag="tnT")
            transpose_to(tn_t, tnT)
            return tnT

        recipTk = npool.tile([P, DC], FP32, tag="recipTk")
        acc_k = kv_attn_phase(NT, recipTk, make_tnT, dump=True)
        if dbg:
            nc.gpsimd.dma_start(out=dext["d_recipk"][:, :], in_=recipTk)

        # scale attn rows by recip_k -> SBUF f32 (drains acc_k)
        attn_sc = npool.tile([P, DC, P], FP32, tag="attn_sc")
        for c in range(DC):
            nc.vector.tensor_scalar_mul(
                out=attn_sc[:, c, :],
                in0=acc_k[c // 4][:, (c % 4) * P:(c % 4 + 1) * P],
                scalar1=recipTk[:, c:c + 1])

        if dbg:
            nc.gpsimd.dma_start(out=dext["d_attnsc"][:, :, :], in_=attn_sc)

        # --- s-path (S = 512 rows: xs -> sn) ---
        def make_snT(st):
            xs_t = work.tile([P, D], FP32, tag="xs_t")
            nc.sync.dma_start(out=xs_t, in_=xs_ext[st * P:(st + 1) * P, :])
            sn_t = work.tile([P, D], BF16, tag="tn_t")
            ln_from_sbuf(xs_t, sn_t, gs_bc, bs_bc)
            snT = work.tile([P, DC, P], BF16, tag="tnT")
            transpose_to(sn_t, snT)
            return snT

        recipTs = npool.tile([P, DC], FP32, tag="recipTs")
        acc_s = kv_attn_phase(ST, recipTs, make_snT)
        if dbg:
            nc.gpsimd.dma_start(out=dext["d_recips"][:, :], in_=recipTs)

        # sattnsum rows: scale G rows by recip_s, mask cross-head terms,
        # then column-sum (mask makes col l sum only over d in head(l)).
        sattn_row = npool.tile([1, DC, P], FP32, tag="sattn_row")
        for c in range(DC):
            gsc = work.tile([P, P], FP32, tag="gsc")
            nc.vector.tensor_scalar_mul(
                out=gsc, in0=acc_s[c // 4][:, (c % 4) * P:(c % 4 + 1) * P],
                scalar1=recipTs[:, c:c + 1])
            nc.vector.tensor_mul(out=gsc, in0=gsc, in1=mask_f)
            ssp = psmall.tile([1, P], FP32, tag="small")
            nc.tensor.matmul(ssp, lhsT=onescol_f, rhs=gsc, start=True, stop=True)
            nc.vector.tensor_copy(out=sattn_row[0:1, c, :], in_=ssp)

        if dbg:
            nc.gpsimd.dma_start(out=dext["d_sattn"][:, :, :], in_=sattn_row)

        # ---------------- attn2 block-diagonal tiles ----------------
        attn2 = npool.tile([P, DC, P], BF16, tag="attn2")
        for c in range(DC):
            psb = psmall.tile([P, P], FP32, tag="small")
            nc.tensor.matmul(psb, lhsT=ones1_f, rhs=sattn_row[0:1, c, :],
                             start=True, stop=True)
            tmp = work.tile([P, P], FP32, tag="a2tmp")
            nc.vector.tensor_add(out=tmp, in0=attn_sc[:, c, :], in1=psb)
            nc.vector.tensor_mul(out=attn2[:, c, :], in0=tmp, in1=mask_f)

        if dbg:
            nc.gpsimd.dma_start(out=dext["d_attn2"][:, :, :], in_=attn2)

        # ---------------- x-path: 32 token tiles ----------------
        for tt in range(TT):
            x_sb = xin.tile([P, D], FP32, tag="x_sb")
            nc.sync.dma_start(out=x_sb, in_=x_ext[tt * P:(tt + 1) * P, :])
            xln = work.tile([P, D], BF16, tag="xln")
            ln_from_sbuf(x_sb, xln, gx_bc, bx_bc)
            xlT = work.tile([P, DC, P], BF16, tag="xlT")
            transpose_to(xln, xlT)
            # q projection + exp
            Eq = work.tile([P, D], FP32, tag="Eq")
            for jh in range(2):
                ps = proj_to_psum(xlT, Wq_sb, bq_r, jh)
                nc.scalar.activation(out=Eq[:, jh * 512:(jh + 1) * 512],
                                     in_=ps, func=AF.Exp)
            # softmax over dh-groups (free dim)
            dsum = work.tile([P, H], FP32, tag="dsum")
            nc.vector.reduce_sum(out=dsum,
                                 in_=Eq.rearrange("p (h d) -> p h d", h=H),
                                 axis=AX)
            nc.vector.reciprocal(out=dsum, in_=dsum)
            qsm = work.tile([P, D], BF16, tag="qsm")
            nc.vector.tensor_mul(
                out=qsm.rearrange("p (h d) -> p h d", h=H),
                in0=Eq.rearrange("p (h d) -> p h d", h=H),
                in1=dsum[:, :, None].broadcast_to([P, H, dh]))
            if dbg:
                nc.gpsimd.dma_start(out=dext["d_eq"][tt], in_=Eq)
                nc.gpsimd.dma_start(out=dext["d_qsm"][tt], in_=qsm)
            qsmT = work.tile([P, DC, P], BF16, tag="qsmT")
            transpose_to(qsm, qsmT)
            # y^T blocks + silu
            siluT = work.tile([P, DC, P], BF16, tag="siluT")
            for g in range(0, DC, 4):
                yps = pproj.tile([P, 512], FP32, tag="proj")
                for k in range(4):
                    c = g + k
                    nc.tensor.matmul(yps[:, k * P:(k + 1) * P],
                                     lhsT=attn2[:, c, :], rhs=qsmT[:, c, :],
                                     start=True, stop=True)
                nc.scalar.activation(
                    out=siluT[:, g:g + 4, :].rearrange("p a b -> p (a b)"),
                    in_=yps, func=AF.Silu)
            if dbg:
                nc.gpsimd.dma_start(out=dext["d_siluT"][tt], in_=siluT)
            # output projection + residual
            o_sb = work.tile([P, D], FP32, tag="o_sb")
            for jh in range(2):
                ps = proj_to_psum(siluT, Wo_sb, bo_r, jh)
                nc.vector.tensor_add(out=o_sb[:, jh * 512:(jh + 1) * 512],
                                     in0=ps, in1=x_sb[:, jh * 512:(jh + 1) * 512])
            nc.sync.dma_start(out=out_ext[tt * P:(tt + 1) * P, :], in_=o_sb)

    nc.compile()
    return nc


def kernel(**inputs) -> np.ndarray:
    from concourse.bass_utils import run_bass_kernel_spmd

    ins = {k: np.ascontiguousarray(np.asarray(v, dtype=np.float32))
           for k, v in inputs.items()}
    affine_x = not (np.all(ins["norm_g"] == 1.0) and np.all(ins["norm_b"] == 0.0))
    affine_t = not (np.all(ins["tnorm_g"] == 1.0) and np.all(ins["tnorm_b"] == 0.0))
    affine_s = not (np.all(ins["snorm_g"] == 1.0) and np.all(ins["snorm_b"] == 0.0))

    key = (affine_x, affine_t, affine_s)
    if key not in _CACHE:
        _CACHE[key] = _build(*key)
    nc = _CACHE[key]

    wnames = ["norm_g", "norm_b", "tnorm_g", "tnorm_b", "snorm_g", "snorm_b",
              "Wq", "bq", "Wk", "bk", "Wv", "bv", "Wa", "ba", "Wat", "bat",
              "Wo", "bo"]
    in_maps = []
    for b in range(NCORES):
        m = {"x": ins["x"][b], "xf": ins["xf"][b], "xw": ins["xw"][b],
             "xs": ins["xs"][b]}
        for nm in wnames:
            m[nm] = ins[nm]
        in_maps.append(m)

    res = run_bass_kernel_spmd(nc, in_maps, core_ids=list(range(NCORES)))
    return np.stack([res.results[i]["out"] for i in range(NCORES)], axis=0)


if __name__ == "__main__":
    import reference
    rin = reference.setup_inputs()
    out = kernel(**{k: np.asarray(v) for k, v in rin.items()})
    print("out shape:", out.shape, out.dtype)


# revision 15
# speedup vs baseline: 1.3114x; 1.3114x over previous
"""Trainium2 Bass kernel for nn_CrossAttention (sparse_attention).

Sharding: data-parallel over B across 8 NeuronCores (1 batch element per
core, weights replicated, no collectives).

Per-core algorithm (T=4096, N=512, S=512, D=1024, H=16, dh=64):
  - exact restructuring: y_tot[t,h,:] = q_sm[t,h,:] @ attn2[h]  where
    attn2[h] = attn[h] + ones(dh) x sattnsum[h]   (exact because
    sum_d q_sm[t,h,d] * sattnsum[h,l] == qsum*sattnsum == reference sy)
  - softmax normalizations deferred: attn[h] = diag(1/colsum(Ek)_h) @ Ek_h^T Vh
    sattnsum[h,l] = sum_d recip_s[h,d] * (Es_h^T Vs_h)[d,l]
  - no softmax max-subtraction (values are small; exp is safe)
  - activations kept token-major in SBUF; PE-transposes feed the matmuls
  - TensorE compute in bf16 (validated fro-rel ~2.3e-3 vs f32 reference)
  - x-path LN stats prepass + tile-pair batching to minimize ACT
    table switches (sqrt/exp/silu live in different ACT table sets)
"""
import numpy as np

H, D, TFD, AUD, EPS = 16, 1024, 256, 768, 1e-5
B, T, N, S = 8, 4096, 512, 512
dh = D // H
P = 128
TT = T // P           # 32 token tiles
NT = 2 * N // P       # 8 n tiles
ST = S // P           # 4 s tiles
DC = D // P           # 8 feature chunks
NCORES = 8

_CACHE = {}


def _build(affine_x, affine_t, affine_s, hasb=None, dbg=False):
    import concourse.bass as bass
    import concourse.tile as tile
    from concourse import bacc, mybir
    from concourse.masks import make_identity

    if hasb is None:
        hasb = {}
    FP32 = mybir.dt.float32
    BF16 = mybir.dt.bfloat16
    AX = mybir.AxisListType.X
    AF = mybir.ActivationFunctionType
    OP = mybir.AluOpType

    nc = bacc.Bacc()

    # ---------------- DRAM parameters (per-core shapes) ----------------
    x_ext = nc.declare_dram_parameter("x", [T, D], FP32, isOutput=False)
    xf_ext = nc.declare_dram_parameter("xf", [AUD], FP32, isOutput=False)
    xw_ext = nc.declare_dram_parameter("xw", [N, TFD], FP32, isOutput=False)
    xs_ext = nc.declare_dram_parameter("xs", [S, D], FP32, isOutput=False)
    wext = {}
    for nm, shp in [
        ("norm_g", [D]), ("norm_b", [D]), ("tnorm_g", [D]), ("tnorm_b", [D]),
        ("snorm_g", [D]), ("snorm_b", [D]),
        ("Wq", [D, D]), ("bq", [D]), ("Wk", [D, D]), ("bk", [D]),
        ("Wv", [D, D]), ("bv", [D]), ("Wa", [AUD, TFD]), ("ba", [TFD]),
        ("Wat", [TFD, D]), ("bat", [D]), ("Wo", [D, D]), ("bo", [D]),
    ]:
        wext[nm] = nc.declare_dram_parameter(nm, shp, FP32, isOutput=False)
    out_ext = nc.declare_dram_parameter("out", [T, D], FP32, isOutput=True)
    dext = {}
    if dbg:
        for nm, shp in [
            ("d_xfpT", [P, 2]), ("d_xcT", [P, 2, 2 * N]),
            ("d_ek", [NT, P, D]), ("d_v", [NT, P, D]),
            ("d_recipk", [P, DC]), ("d_recips", [P, DC]),
            ("d_attnsc", [P, DC, P]), ("d_sattn", [1, DC, P]),
            ("d_attn2", [P, DC, P]), ("d_eq", [TT, P, D]),
            ("d_qsm", [TT, P, D]), ("d_siluT", [TT, P, DC, P]),
        ]:
            dext[nm] = nc.declare_dram_parameter(nm, shp, FP32, isOutput=True)

    ctx_pools = {}

    with tile.TileContext(nc) as tc, \
         tc.tile_pool(name="wpool", bufs=1) as wpool, \
         tc.tile_pool(name="npool", bufs=1) as npool, \
         tc.tile_pool(name="work", bufs=2) as work, \
         tc.tile_pool(name="chain", bufs=4) as chain, \
         tc.tile_pool(name="xin", bufs=3) as xin, \
         tc.tile_pool(name="pproj", bufs=2, space="PSUM") as pproj, \
         tc.tile_pool(name="ptp", bufs=2, space="PSUM") as ptp:

        # ---------------- constants ----------------
        ident_bf = wpool.tile([P, P], BF16, tag="ident_bf")
        make_identity(nc, ident_bf)
        ones1_bf = wpool.tile([1, P], BF16, tag="ones1_bf")
        nc.vector.memset(ones1_bf, 1.0)
        ones1_f = wpool.tile([1, P], FP32, tag="ones1_f")
        nc.vector.memset(ones1_f, 1.0)
        onescol_bf = wpool.tile([P, 1], BF16, tag="onescol_bf")
        nc.vector.memset(onescol_bf, 1.0)
        onescol_f = wpool.tile([P, 1], FP32, tag="onescol_f")
        nc.vector.memset(onescol_f, 1.0)
        mask_f = wpool.tile([P, P], FP32, tag="mask_f")
        nc.vector.memset(mask_f, 0.0)
        nc.vector.memset(mask_f[0:dh, 0:dh], 1.0)
        nc.vector.memset(mask_f[dh:P, dh:P], 1.0)
        eps_t = wpool.tile([P, 1], FP32, tag="eps_t")
        nc.vector.memset(eps_t, EPS)

        # ---------------- weights (DMA-cast f32 -> bf16) ----------------
        def load_w(nm, rows, cols):
            t = wpool.tile([P, rows // P, cols], BF16, tag=nm)
            nc.gpsimd.dma_start(
                out=t, in_=wext[nm][:, :].rearrange("(c p) n -> p c n", p=P))
            return t

        def load_row(nm, L):
            if not hasb.get(nm, True):
                return None
            t = wpool.tile([1, L], BF16, tag=nm + "_r")
            nc.gpsimd.dma_start(out=t, in_=wext[nm][:][None, :])
            return t

        # order matters: first-needed weights first so PE starts early
        xf_col = wpool.tile([P, AUD // P], BF16, tag="xf_col")
        nc.gpsimd.dma_start(out=xf_col,
                            in_=xf_ext[:].rearrange("(c p) -> p c", p=P))
        Wa_sb = load_w("Wa", AUD, TFD)
        Wat_sb = load_w("Wat", TFD, D)
        Wk_sb = load_w("Wk", D, D)
        Wv_sb = load_w("Wv", D, D)
        Wq_sb = load_w("Wq", D, D)
        Wo_sb = load_w("Wo", D, D)
        ba_r = load_row("ba", TFD)
        bat_r = load_row("bat", D)
        bk_r = load_row("bk", D)
        bv_r = load_row("bv", D)
        bq_r = load_row("bq", D)
        bo_r = load_row("bo", D)

        def bcast_vec(nm):
            t = wpool.tile([P, D], FP32, tag=nm + "_bc")
            src = wext[nm][:][None, :].broadcast_to([P, D])
            nc.gpsimd.dma_start(out=t, in_=src)
            return t

        gx_bc = bcast_vec("norm_g") if affine_x else None
        bx_bc = bcast_vec("norm_b") if affine_x else None
        gt_bc = bcast_vec("tnorm_g") if affine_t else None
        bt_bc = bcast_vec("tnorm_b") if affine_t else None
        gs_bc = bcast_vec("snorm_g") if affine_s else None
        bs_bc = bcast_vec("snorm_b") if affine_s else None

        # ---------------- shared helpers ----------------
        def ln_stats(src_aps, mv_out):
            """bn stats over free-dim halves -> mv_out [P,2] = mean, var."""
            stats = work.tile([P, len(src_aps), 6], FP32, tag="stats")
            for j, ap in enumerate(src_aps):
                nc.vector.bn_stats(out=stats[:, j, :], in_=ap)
            nc.vector.bn_aggr(out=mv_out, in_=stats)

        def rstd_inplace(var_ap):
            nc.scalar.activation(out=var_ap, in_=var_ap,
                                 func=AF.Sqrt, bias=eps_t, scale=1.0)
            nc.vector.reciprocal(out=var_ap, in_=var_ap)

        def ln_apply(src_ap, dst_ap, mean_ap, rstd_ap, g_bc, b_bc, gslc=None):
            if g_bc is None:
                nc.vector.tensor_scalar(
                    out=dst_ap, in0=src_ap, scalar1=mean_ap, scalar2=rstd_ap,
                    op0=OP.subtract, op1=OP.mult)
            else:
                tmpf = work.tile([P, 512], FP32, tag="lnt")
                sl = tmpf[:, 0:src_ap.free_size()]
                nc.vector.tensor_scalar(
                    out=sl, in0=src_ap, scalar1=mean_ap, scalar2=rstd_ap,
                    op0=OP.subtract, op1=OP.mult)
                nc.vector.tensor_mul(out=sl, in0=sl, in1=g_bc[:, gslc])
                nc.vector.tensor_add(out=dst_ap, in0=sl, in1=b_bc[:, gslc])

        def transpose_to(src_bf, dstT):
            """src_bf [P, nchunk*P] token-major bf16 -> dstT [P, nchunk, P]."""
            nchunk = src_bf.shape[-1] // P
            for g in range(0, nchunk, 4):
                cnt = min(4, nchunk - g)
                tps = ptp.tile([P, 512], BF16, tag="tpbf")
                for k in range(cnt):
                    c = g + k
                    nc.tensor.transpose(tps[:, k * P:(k + 1) * P],
                                        src_bf[:, c * P:(c + 1) * P], ident_bf)
                nc.vector.tensor_copy(
                    out=dstT[:, g:g + cnt, :],
                    in_=tps[:, 0:cnt * P].rearrange("p (a b) -> p a b", a=cnt))

        def proj_to_psum(actT, w_sb, b_r, jh):
            ps = pproj.tile([P, 512], FP32, tag="proj")
            for dc in range(DC):
                nc.tensor.matmul(ps, lhsT=actT[:, dc, :],
                                 rhs=w_sb[:, dc, jh * 512:(jh + 1) * 512],
                                 start=(dc == 0), stop=(b_r is None and dc == DC - 1))
            if b_r is not None:
                nc.tensor.matmul(ps, lhsT=ones1_bf,
                                 rhs=b_r[0:1, jh * 512:(jh + 1) * 512],
                                 start=False, stop=True)
            return ps

        # ---------------- n1: xf_projT [P, 2] f32 ----------------
        with tc.tile_pool(name="pacc", bufs=2, space="PSUM") as pacc, \
             tc.tile_pool(name="psmall", bufs=2, space="PSUM") as psmall:

            xfpT = npool.tile([P, 2], FP32, tag="xfpT")
            for m in range(2):
                ps = psmall.tile([P, 1], FP32, tag="small")
                nmm = AUD // P
                for ac in range(nmm):
                    nc.tensor.matmul(ps, lhsT=Wa_sb[:, ac, m * P:(m + 1) * P],
                                     rhs=xf_col[:, ac:ac + 1],
                                     start=(ac == 0),
                                     stop=(ba_r is None and ac == nmm - 1))
                if ba_r is not None:
                    nc.tensor.matmul(ps, lhsT=ba_r[0:1, m * P:(m + 1) * P],
                                     rhs=ones1_bf[0:1, 0:1], start=False,
                                     stop=True)
                nc.vector.tensor_copy(out=xfpT[:, m:m + 1], in_=ps)
            if dbg:
                nc.gpsimd.dma_start(out=dext["d_xfpT"][:, :], in_=xfpT)

            # ---------------- n2: xcT [P, 2, 2N] bf16 ----------------
            xcT = npool.tile([P, 2, 2 * N], BF16, tag="xcT")
            for nt in range(N // P):
                xw_t = work.tile([P, TFD], BF16, tag="xw_t")
                nc.gpsimd.dma_start(out=xw_t, in_=xw_ext[nt * P:(nt + 1) * P, :])
                for tc2 in range(2):
                    tp = ptp.tile([P, P], BF16, tag="tpbf")
                    nc.tensor.transpose(tp, xw_t[:, tc2 * P:(tc2 + 1) * P],
                                        ident_bf)
                    nc.vector.tensor_copy(out=xcT[:, tc2, nt * P:(nt + 1) * P],
                                          in_=tp)
            for tc2 in range(2):
                nc.vector.tensor_copy(out=xcT[:, tc2, N:2 * N],
                                      in_=xfpT[:, tc2:tc2 + 1].broadcast_to([P, N]))
            if dbg:
                nc.gpsimd.dma_start(out=dext["d_xcT"][:, :, :], in_=xcT)

            # ------------- K/V + attn/denominator accumulation -------------
            def kv_attn_phase(nseq_tiles, recip_dst, make_actT, dump=False):
                acc0 = pacc.tile([P, 512], FP32, tag="acc")
                acc1 = pacc.tile([P, 512], FP32, tag="acc")
                acc = [acc0, acc1]
                dT = psmall.tile([P, DC], FP32, tag="small")
                # Interleaved accumulation chains share banks; start=True
                # would clear the whole bank's has_written bits and drop
                # sibling chains' partials. Zero the banks, accumulate with
                # start=False (overwrite-where-clear).
                nc.vector.memset(acc0, 0.0)
                nc.vector.memset(acc1, 0.0)
                nc.vector.memset(dT, 0.0)
                for it in range(nseq_tiles):
                    actT = make_actT(it)
                    ek = work.tile([P, D], BF16, tag="ek_t")
                    vv = work.tile([P, D], BF16, tag="v_t")
                    for w_sb, b_r, is_k in ((Wk_sb, bk_r, True),
                                            (Wv_sb, bv_r, False)):
                        for jh in range(2):
                            ps = proj_to_psum(actT, w_sb, b_r, jh)
                            if is_k:
                                nc.scalar.activation(
                                    out=ek[:, jh * 512:(jh + 1) * 512],
                                    in_=ps, func=AF.Exp)
                            else:
                                nc.vector.tensor_copy(
                                    out=vv[:, jh * 512:(jh + 1) * 512], in_=ps)
                    if dbg and dump:
                        nc.gpsimd.dma_start(out=dext["d_ek"][it], in_=ek)
                        nc.gpsimd.dma_start(out=dext["d_v"][it], in_=vv)
                    last = it == nseq_tiles - 1
                    for c in range(DC):
                        nc.tensor.matmul(
                            acc[c // 4][:, (c % 4) * P:(c % 4 + 1) * P],
                            lhsT=ek[:, c * P:(c + 1) * P],
                            rhs=vv[:, c * P:(c + 1) * P],
                            start=False, stop=last, skip_group_check=True)
                    for dc in range(DC):
                        nc.tensor.matmul(
                            dT[:, dc:dc + 1],
                            lhsT=ek[:, dc * P:(dc + 1) * P],
                            rhs=onescol_bf,
                            start=False, stop=last, skip_group_check=True)
                nc.vector.reciprocal(out=recip_dst, in_=dT)
                return acc

            # --- n-path (2N = 1024 rows: xw_xf -> tn) ---
            def make_tnT(nt):
                psa = pproj.tile([P, 512], FP32, tag="proj")
                psb = pproj.tile([P, 512], FP32, tag="proj")
                for jh, ps in enumerate((psa, psb)):
                    for tc2 in range(2):
                        nc.tensor.matmul(
                            ps, lhsT=xcT[:, tc2, nt * P:(nt + 1) * P],
                            rhs=Wat_sb[:, tc2, jh * 512:(jh + 1) * 512],
                            start=(tc2 == 0),
                            stop=(bat_r is None and tc2 == 1))
                    if bat_r is not None:
                        nc.tensor.matmul(ps, lhsT=ones1_bf,
                                         rhs=bat_r[0:1, jh * 512:(jh + 1) * 512],
                                         start=False, stop=True)
                mv = work.tile([P, 2], FP32, tag="mv")
                ln_stats((psa, psb), mv)
                rstd_inplace(mv[:, 1:2])
                tn_t = work.tile([P, D], BF16, tag="tn_t")
                for j, ps in enumerate((psa, psb)):
                    ln_apply(ps, tn_t[:, j * 512:(j + 1) * 512],
                             mv[:, 0:1], mv[:, 1:2], gt_bc, bt_bc,
                             slice(j * 512, (j + 1) * 512))
                tnT = work.tile([P, DC, P], BF16, tag="tnT")
                transpose_to(tn_t, tnT)
                return tnT

            recipTk = npool.tile([P, DC], FP32, tag="recipTk")
            acc_k = kv_attn_phase(NT, recipTk, make_tnT, dump=True)
            if dbg:
                nc.gpsimd.dma_start(out=dext["d_recipk"][:, :], in_=recipTk)

            # scale attn rows by recip_k -> SBUF f32 (drains acc_k)
            attn_sc = npool.tile([P, DC, P], FP32, tag="attn_sc")
            for c in range(DC):
                nc.vector.tensor_scalar_mul(
                    out=attn_sc[:, c, :],
                    in0=acc_k[c // 4][:, (c % 4) * P:(c % 4 + 1) * P],
                    scalar1=recipTk[:, c:c + 1])
            if dbg:
                nc.gpsimd.dma_start(out=dext["d_attnsc"][:, :, :], in_=attn_sc)

            # --- s-path (S = 512 rows: xs -> sn) ---
            def make_snT(st):
                xs_t = work.tile([P, D], FP32, tag="xs_t")
                nc.sync.dma_start(out=xs_t, in_=xs_ext[st * P:(st + 1) * P, :])
                mv = work.tile([P, 2], FP32, tag="mv")
                ln_stats((xs_t[:, 0:512], xs_t[:, 512:1024]), mv)
                rstd_inplace(mv[:, 1:2])
                sn_t = work.tile([P, D], BF16, tag="tn_t")
                if gs_bc is None:
                    nc.vector.tensor_scalar(
                        out=sn_t, in0=xs_t, scalar1=mv[:, 0:1],
                        scalar2=mv[:, 1:2], op0=OP.subtract, op1=OP.mult)
                else:
                    for j in range(2):
                        ln_apply(xs_t[:, j * 512:(j + 1) * 512],
                                 sn_t[:, j * 512:(j + 1) * 512],
                                 mv[:, 0:1], mv[:, 1:2], gs_bc, bs_bc,
                                 slice(j * 512, (j + 1) * 512))
                snT = work.tile([P, DC, P], BF16, tag="tnT")
                transpose_to(sn_t, snT)
                return snT

            recipTs = npool.tile([P, DC], FP32, tag="recipTs")
            acc_s = kv_attn_phase(ST, recipTs, make_snT)
            if dbg:
                nc.gpsimd.dma_start(out=dext["d_recips"][:, :], in_=recipTs)

            # sattnsum rows: scale G rows by recip_s, mask cross-head
            # terms, column-sum (col l then sums only d in head(l)).
            sattn_row = npool.tile([1, DC, P], FP32, tag="sattn_row")
            for c in range(DC):
                gsc = work.tile([P, P], FP32, tag="gsc")
                nc.vector.tensor_scalar_mul(
                    out=gsc, in0=acc_s[c // 4][:, (c % 4) * P:(c % 4 + 1) * P],
                    scalar1=recipTs[:, c:c + 1])
                nc.vector.tensor_mul(out=gsc, in0=gsc, in1=mask_f)
                ssp = psmall.tile([1, P], FP32, tag="small")
                nc.tensor.matmul(ssp, lhsT=onescol_f, rhs=gsc,
                                 start=True, stop=True)
                nc.vector.tensor_copy(out=sattn_row[0:1, c, :], in_=ssp)
            if dbg:
                nc.gpsimd.dma_start(out=dext["d_sattn"][:, :, :], in_=sattn_row)

            # ---------------- attn2 block-diagonal tiles ----------------
            attn2 = npool.tile([P, DC, P], BF16, tag="attn2")
            for c in range(DC):
                psb = psmall.tile([P, P], FP32, tag="small")
                nc.tensor.matmul(psb, lhsT=ones1_f, rhs=sattn_row[0:1, c, :],
                                 start=True, stop=True)
                tmp = work.tile([P, P], FP32, tag="a2tmp")
                nc.vector.tensor_add(out=tmp, in0=attn_sc[:, c, :], in1=psb)
                nc.vector.tensor_mul(out=attn2[:, c, :], in0=tmp, in1=mask_f)
            if dbg:
                nc.gpsimd.dma_start(out=dext["d_attn2"][:, :, :], in_=attn2)

        # ---------------- x-path LN stats prepass ----------------
        # One Sqrt for all 32 tiles: keeps the main loop's ACT usage to
        # exp/silu only (fewer ACT table switches).
        mvall = npool.tile([P, TT, 2], FP32, tag="mvall")
        for tt in range(TT):
            x_pre = xin.tile([P, D], FP32, tag="x_pre")
            nc.sync.dma_start(out=x_pre, in_=x_ext[tt * P:(tt + 1) * P, :])
            ln_stats((x_pre[:, 0:512], x_pre[:, 512:1024]), mvall[:, tt, :])
        nc.scalar.activation(out=mvall[:, :, 1], in_=mvall[:, :, 1],
                             func=AF.Sqrt, bias=eps_t, scale=1.0)
        nc.vector.reciprocal(out=mvall[:, :, 1], in_=mvall[:, :, 1])

        # ---------------- x-path: 32 token tiles, paired ----------------
        with tc.tile_pool(name="pxtra", bufs=2, space="PSUM") as pxtra:

            def stage1(tt):
                """load, LN, transpose, q-proj, exp -> (x_sb, Eq)"""
                x_sb = xin.tile([P, D], FP32, tag="x_sb")
                nc.sync.dma_start(out=x_sb, in_=x_ext[tt * P:(tt + 1) * P, :])
                xln = chain.tile([P, D], BF16, tag="xln")
                if gx_bc is None:
                    nc.vector.tensor_scalar(
                        out=xln, in0=x_sb, scalar1=mvall[:, tt, 0:1],
                        scalar2=mvall[:, tt, 1:2], op0=OP.subtract, op1=OP.mult)
                else:
                    for j in range(2):
                        ln_apply(x_sb[:, j * 512:(j + 1) * 512],
                                 xln[:, j * 512:(j + 1) * 512],
                                 mvall[:, tt, 0:1], mvall[:, tt, 1:2],
                                 gx_bc, bx_bc, slice(j * 512, (j + 1) * 512))
                xlT = chain.tile([P, DC, P], BF16, tag="xlT")
                transpose_to(xln, xlT)
                Eq = chain.tile([P, D], BF16, tag="Eq")
                for jh in range(2):
                    ps = proj_to_psum(xlT, Wq_sb, bq_r, jh)
                    nc.scalar.activation(out=Eq[:, jh * 512:(jh + 1) * 512],
                                         in_=ps, func=AF.Exp)
                return x_sb, Eq

            def stage2(tt, Eq):
                """softmax over dh-groups -> qsmT"""
                dsum = work.tile([P, H], FP32, tag="dsum")
                nc.vector.reduce_sum(out=dsum,
                                     in_=Eq.rearrange("p (h d) -> p h d", h=H),
                                     axis=AX)
                nc.vector.reciprocal(out=dsum, in_=dsum)
                dsum_bf = work.tile([P, H], BF16, tag="dsum_bf")
                nc.vector.tensor_copy(out=dsum_bf, in_=dsum)
                qsm = chain.tile([P, D], BF16, tag="qsm")
                nc.vector.tensor_mul(
                    out=qsm.rearrange("p (h d) -> p h d", h=H),
                    in0=Eq.rearrange("p (h d) -> p h d", h=H),
                    in1=dsum_bf[:, :, None].broadcast_to([P, H, dh]))
                if dbg:
                    nc.gpsimd.dma_start(out=dext["d_eq"][tt], in_=Eq)
                    nc.gpsimd.dma_start(out=dext["d_qsm"][tt], in_=qsm)
                qsmT = chain.tile([P, DC, P], BF16, tag="qsmT")
                transpose_to(qsm, qsmT)
                return qsmT

            def stage3(tt, qsmT):
                """y^T blocks + silu -> siluT"""
                siluT = chain.tile([P, DC, P], BF16, tag="siluT")
                for g in range(0, DC, 4):
                    yps = pxtra.tile([P, 512], FP32, tag="ypsum")
                    for k in range(4):
                        c = g + k
                        nc.tensor.matmul(yps[:, k * P:(k + 1) * P],
                                         lhsT=attn2[:, c, :], rhs=qsmT[:, c, :],
                                         start=True, stop=True)
                    nc.scalar.activation(
                        out=siluT[:, g:g + 4, :].rearrange("p a b -> p (a b)"),
                        in_=yps, func=AF.Silu)
                if dbg:
                    nc.gpsimd.dma_start(out=dext["d_siluT"][tt], in_=siluT)
                return siluT

            def stage4(tt, siluT, x_sb):
                """out-proj + residual + store"""
                o_sb = chain.tile([P, D], FP32, tag="o_sb")
                for jh in range(2):
                    ps = pxtra.tile([P, 512], FP32, tag="opsum")
                    for c in range(DC):
                        nc.tensor.matmul(
                            ps, lhsT=siluT[:, c, :],
                            rhs=Wo_sb[:, c, jh * 512:(jh + 1) * 512],
                            start=(c == 0),
                            stop=(bo_r is None and c == DC - 1))
                    if bo_r is not None:
                        nc.tensor.matmul(ps, lhsT=ones1_bf,
                                         rhs=bo_r[0:1, jh * 512:(jh + 1) * 512],
                                         start=False, stop=True)
                    nc.vector.tensor_add(
                        out=o_sb[:, jh * 512:(jh + 1) * 512], in0=ps,
                        in1=x_sb[:, jh * 512:(jh + 1) * 512])
                nc.sync.dma_start(out=out_ext[tt * P:(tt + 1) * P, :], in_=o_sb)

            for pair in range(TT // 2):
                t0, t1 = 2 * pair, 2 * pair + 1
                xa, Ea = stage1(t0)
                xb, Eb = stage1(t1)
                qa = stage2(t0, Ea)
                qb = stage2(t1, Eb)
                sa = stage3(t0, qa)
                sb = stage3(t1, qb)
                stage4(t0, sa, xa)
                stage4(t1, sb, xb)

    nc.compile()
    return nc


def kernel(**inputs) -> np.ndarray:
    from concourse.bass_utils import run_bass_kernel_spmd

    ins = {k: np.ascontiguousarray(np.asarray(v, dtype=np.float32))
           for k, v in inputs.items()}
    affine_x = not (np.all(ins["norm_g"] == 1.0) and np.all(ins["norm_b"] == 0.0))
    affine_t = not (np.all(ins["tnorm_g"] == 1.0) and np.all(ins["tnorm_b"] == 0.0))
    affine_s = not (np.all(ins["snorm_g"] == 1.0) and np.all(ins["snorm_b"] == 0.0))
    hasb = {nm: bool(np.any(ins[nm] != 0.0))
            for nm in ("bq", "bk", "bv", "ba", "bat", "bo")}

    key = (affine_x, affine_t, affine_s, tuple(sorted(hasb.items())))
    if key not in _CACHE:
        _CACHE[key] = _build(affine_x, affine_t, affine_s, hasb)
    nc = _CACHE[key]

    wnames = ["norm_g", "norm_b", "tnorm_g", "tnorm_b", "snorm_g", "snorm_b",
              "Wq", "bq", "Wk", "bk", "Wv", "bv", "Wa", "ba", "Wat", "bat",
              "Wo", "bo"]
    in_maps = []
    for b in range(NCORES):
        m = {"x": ins["x"][b], "xf": ins["xf"][b], "xw": ins["xw"][b],
             "xs": ins["xs"][b]}
        for nm in wnames:
            m[nm] = ins[nm]
        in_maps.append(m)

    res = run_bass_kernel_spmd(nc, in_maps, core_ids=list(range(NCORES)))
    return np.stack([res.results[i]["out"] for i in range(NCORES)], axis=0)


if __name__ == "__main__":
    import reference
    rin = reference.setup_inputs()
    out = kernel(**{k: np.asarray(v) for k, v in rin.items()})
    print("out shape:", out.shape, out.dtype)


# revision 22
# speedup vs baseline: 1.3662x; 1.0418x over previous
"""Trainium2 Bass kernel for nn_CrossAttention (sparse_attention).

Sharding: data-parallel over B across 8 NeuronCores (1 batch element per
core, weights replicated, no collectives).

Per-core algorithm (T=4096, N=512, S=512, D=1024, H=16, dh=64):
  - exact restructuring: y_tot[t,h,:] = q_sm[t,h,:] @ attn2[h]  where
    attn2[h] = attn[h] + ones(dh) x sattnsum[h]   (exact because
    sum_d q_sm[t,h,d] * sattnsum[h,l] == qsum*sattnsum == reference sy)
  - softmax normalizations deferred: attn[h] = diag(1/colsum(Ek)_h) @ Ek_h^T Vh
    sattnsum[h,l] = sum_d recip_s[h,d] * (Es_h^T Vs_h)[d,l]
  - no softmax max-subtraction (values are small; exp is safe)
  - activations kept token-major in SBUF; PE-transposes feed the matmuls
  - TensorE compute in bf16 (validated fro-rel ~2.3e-3 vs f32 reference)
  - x-path LN stats prepass + tile-pair batching to minimize ACT
    table switches (sqrt/exp/silu live in different ACT table sets)
"""
import numpy as np

H, D, TFD, AUD, EPS = 16, 1024, 256, 768, 1e-5
B, T, N, S = 8, 4096, 512, 512
dh = D // H
P = 128
TT = T // P           # 32 token tiles
NT = 2 * N // P       # 8 n tiles
ST = S // P           # 4 s tiles
DC = D // P           # 8 feature chunks
NCORES = 8

_CACHE = {}


def _build(affine_x, affine_t, affine_s, hasb=None, dbg=False):
    import concourse.bass as bass
    import concourse.tile as tile
    from concourse import bacc, mybir
    from concourse.masks import make_identity

    if hasb is None:
        hasb = {}
    FP32 = mybir.dt.float32
    BF16 = mybir.dt.bfloat16
    AX = mybir.AxisListType.X
    AF = mybir.ActivationFunctionType
    OP = mybir.AluOpType

    nc = bacc.Bacc()

    # ---------------- DRAM parameters (per-core shapes) ----------------
    x_ext = nc.declare_dram_parameter("x", [T, D], FP32, isOutput=False)
    xf_ext = nc.declare_dram_parameter("xf", [AUD], FP32, isOutput=False)
    xw_ext = nc.declare_dram_parameter("xw", [N, TFD], FP32, isOutput=False)
    xs_ext = nc.declare_dram_parameter("xs", [S, D], FP32, isOutput=False)
    wext = {}
    for nm, shp in [
        ("norm_g", [D]), ("norm_b", [D]), ("tnorm_g", [D]), ("tnorm_b", [D]),
        ("snorm_g", [D]), ("snorm_b", [D]),
        ("Wq", [D, D]), ("bq", [D]), ("Wk", [D, D]), ("bk", [D]),
        ("Wv", [D, D]), ("bv", [D]), ("Wa", [AUD, TFD]), ("ba", [TFD]),
        ("Wat", [TFD, D]), ("bat", [D]), ("Wo", [D, D]), ("bo", [D]),
    ]:
        wext[nm] = nc.declare_dram_parameter(nm, shp, FP32, isOutput=False)
    out_ext = nc.declare_dram_parameter("out", [T, D], FP32, isOutput=True)
    dext = {}
    if dbg:
        for nm, shp in [
            ("d_xfpT", [P, 2]), ("d_xcT", [P, 2, 2 * N]),
            ("d_ek", [NT, P, D]), ("d_v", [NT, P, D]),
            ("d_recipk", [P, DC]), ("d_recips", [P, DC]),
            ("d_attnsc", [P, DC, P]), ("d_sattn", [1, DC, P]),
            ("d_attn2", [P, DC, P]), ("d_eq", [TT, P, D]),
            ("d_qsm", [TT, P, D]), ("d_siluT", [TT, P, DC, P]),
        ]:
            dext[nm] = nc.declare_dram_parameter(nm, shp, FP32, isOutput=True)

    ctx_pools = {}

    with tile.TileContext(nc) as tc, \
         tc.tile_pool(name="wpool", bufs=1) as wpool, \
         tc.tile_pool(name="npool", bufs=1) as npool, \
         tc.tile_pool(name="work", bufs=2) as work, \
         tc.tile_pool(name="chain", bufs=4) as chain, \
         tc.tile_pool(name="xin", bufs=3) as xin, \
         tc.tile_pool(name="pproj", bufs=2, space="PSUM") as pproj, \
         tc.tile_pool(name="ptp", bufs=2, space="PSUM") as ptp:

        # ---------------- constants ----------------
        ident_bf = wpool.tile([P, P], BF16, tag="ident_bf")
        make_identity(nc, ident_bf)
        ones1_bf = wpool.tile([1, P], BF16, tag="ones1_bf")
        nc.vector.memset(ones1_bf, 1.0)
        ones1_f = wpool.tile([1, P], FP32, tag="ones1_f")
        nc.vector.memset(ones1_f, 1.0)
        onescol_bf = wpool.tile([P, 1], BF16, tag="onescol_bf")
        nc.vector.memset(onescol_bf, 1.0)
        onescol_f = wpool.tile([P, 1], FP32, tag="onescol_f")
        nc.vector.memset(onescol_f, 1.0)
        mask_f = wpool.tile([P, P], FP32, tag="mask_f")
        nc.vector.memset(mask_f, 0.0)
        nc.vector.memset(mask_f[0:dh, 0:dh], 1.0)
        nc.vector.memset(mask_f[dh:P, dh:P], 1.0)
        eps_t = wpool.tile([P, 1], FP32, tag="eps_t")
        nc.vector.memset(eps_t, EPS)
        ln512_t = wpool.tile([1, 1], FP32, tag="ln512_t")
        nc.vector.memset(ln512_t, float(np.log(N)))

        # ---------------- weights (DMA-cast f32 -> bf16) ----------------
        def load_w(nm, rows, cols):
            t = wpool.tile([P, rows // P, cols], BF16, tag=nm)
            src = wext[nm][:, :].rearrange("(c p) n -> p c n", p=P)
            for c in range(rows // P):
                nc.gpsimd.dma_start(out=t[:, c, :], in_=src[:, c, :])
            return t

        def load_row(nm, L):
            if not hasb.get(nm, True):
                return None
            t = wpool.tile([1, L], BF16, tag=nm + "_r")
            nc.gpsimd.dma_start(out=t, in_=wext[nm][:][None, :])
            return t

        # order matters: first-needed weights first so PE starts early
        xf_col = wpool.tile([P, AUD // P], BF16, tag="xf_col")
        nc.gpsimd.dma_start(out=xf_col,
                            in_=xf_ext[:].rearrange("(c p) -> p c", p=P))
        Wa_sb = load_w("Wa", AUD, TFD)
        Wat_sb = load_w("Wat", TFD, D)
        Wk_sb = load_w("Wk", D, D)
        Wv_sb = load_w("Wv", D, D)
        Wq_sb = load_w("Wq", D, D)
        Wo_sb = load_w("Wo", D, D)
        ba_r = load_row("ba", TFD)
        bat_r = load_row("bat", D)
        bk_r = load_row("bk", D)
        bv_r = load_row("bv", D)
        bq_r = load_row("bq", D)
        bo_r = load_row("bo", D)

        def bcast_vec(nm):
            t = wpool.tile([P, D], FP32, tag=nm + "_bc")
            src = wext[nm][:][None, :].broadcast_to([P, D])
            nc.gpsimd.dma_start(out=t, in_=src)
            return t

        gx_bc = bcast_vec("norm_g") if affine_x else None
        bx_bc = bcast_vec("norm_b") if affine_x else None
        gt_bc = bcast_vec("tnorm_g") if affine_t else None
        bt_bc = bcast_vec("tnorm_b") if affine_t else None
        gs_bc = bcast_vec("snorm_g") if affine_s else None
        bs_bc = bcast_vec("snorm_b") if affine_s else None

        # ---------------- shared helpers ----------------
        def ln_stats(src_aps, mv_out):
            """bn stats over free-dim halves -> mv_out [P,2] = mean, var."""
            stats = work.tile([P, len(src_aps), 6], FP32, tag="stats")
            for j, ap in enumerate(src_aps):
                nc.vector.bn_stats(out=stats[:, j, :], in_=ap)
            nc.vector.bn_aggr(out=mv_out, in_=stats)

        def rstd_inplace(var_ap):
            nc.scalar.activation(out=var_ap, in_=var_ap,
                                 func=AF.Sqrt, bias=eps_t, scale=1.0)
            nc.vector.reciprocal(out=var_ap, in_=var_ap)

        def ln_apply(src_ap, dst_ap, mean_ap, rstd_ap, g_bc, b_bc, gslc=None):
            if g_bc is None:
                nc.vector.tensor_scalar(
                    out=dst_ap, in0=src_ap, scalar1=mean_ap, scalar2=rstd_ap,
                    op0=OP.subtract, op1=OP.mult)
            else:
                tmpf = work.tile([P, 512], FP32, tag="lnt")
                sl = tmpf[:, 0:src_ap.free_size()]
                nc.vector.tensor_scalar(
                    out=sl, in0=src_ap, scalar1=mean_ap, scalar2=rstd_ap,
                    op0=OP.subtract, op1=OP.mult)
                nc.vector.tensor_mul(out=sl, in0=sl, in1=g_bc[:, gslc])
                nc.vector.tensor_add(out=dst_ap, in0=sl, in1=b_bc[:, gslc])

        def transpose_to(src_bf, dstT, copy_eng=None):
            """src_bf [P, nchunk*P] token-major bf16 -> dstT [P, nchunk, P]."""
            nchunk = src_bf.shape[-1] // P
            for g in range(0, nchunk, 4):
                cnt = min(4, nchunk - g)
                tps = ptp.tile([P, 512], BF16, tag="tpbf")
                for k in range(cnt):
                    c = g + k
                    nc.tensor.transpose(tps[:, k * P:(k + 1) * P],
                                        src_bf[:, c * P:(c + 1) * P], ident_bf)
                src = tps[:, 0:cnt * P].rearrange("p (a b) -> p a b", a=cnt)
                if copy_eng == "scalar":
                    nc.scalar.copy(out=dstT[:, g:g + cnt, :], in_=src)
                else:
                    nc.vector.tensor_copy(out=dstT[:, g:g + cnt, :], in_=src)

        def proj_to_psum(actT, w_sb, b_r, jh):
            ps = pproj.tile([P, 512], FP32, tag="proj")
            for dc in range(DC):
                nc.tensor.matmul(ps, lhsT=actT[:, dc, :],
                                 rhs=w_sb[:, dc, jh * 512:(jh + 1) * 512],
                                 start=(dc == 0), stop=(b_r is None and dc == DC - 1))
            if b_r is not None:
                nc.tensor.matmul(ps, lhsT=ones1_bf,
                                 rhs=b_r[0:1, jh * 512:(jh + 1) * 512],
                                 start=False, stop=True)
            return ps

        # ---------------- x-path LN stats prepass ----------------
        # One Sqrt for all 32 tiles: keeps the main loop's ACT usage to
        # exp/silu only (fewer ACT table switches).
        mvall = npool.tile([P, TT, 2], FP32, tag="mvall")
        for tt in range(TT):
            x_pre = xin.tile([P, D], FP32, tag="x_sb")
            nc.sync.dma_start(out=x_pre, in_=x_ext[tt * P:(tt + 1) * P, :])
            ln_stats((x_pre[:, 0:512], x_pre[:, 512:1024]), mvall[:, tt, :])
        nc.scalar.activation(out=mvall[:, :, 1], in_=mvall[:, :, 1],
                             func=AF.Sqrt, bias=eps_t, scale=1.0)
        nc.vector.reciprocal(out=mvall[:, :, 1], in_=mvall[:, :, 1])

        # ---------------- n1: xf_projT [P, 2] f32 ----------------
        with tc.tile_pool(name="pacc", bufs=2, space="PSUM") as pacc, \
             tc.tile_pool(name="psmall", bufs=2, space="PSUM") as psmall:

            xfpT = npool.tile([P, 2], FP32, tag="xfpT")
            for m in range(2):
                ps = psmall.tile([P, 1], FP32, tag="small")
                nmm = AUD // P
                for ac in range(nmm):
                    nc.tensor.matmul(ps, lhsT=Wa_sb[:, ac, m * P:(m + 1) * P],
                                     rhs=xf_col[:, ac:ac + 1],
                                     start=(ac == 0),
                                     stop=(ba_r is None and ac == nmm - 1))
                if ba_r is not None:
                    nc.tensor.matmul(ps, lhsT=ba_r[0:1, m * P:(m + 1) * P],
                                     rhs=ones1_bf[0:1, 0:1], start=False,
                                     stop=True)
                nc.vector.tensor_copy(out=xfpT[:, m:m + 1], in_=ps)
            if dbg:
                nc.gpsimd.dma_start(out=dext["d_xfpT"][:, :], in_=xfpT)

            # ---------------- n2: xcT [P, 2, N] bf16 (xw half only;
            # rows N..2N are all the xf_proj row -> handled as a rank-1
            # "rep row" contribution scaled by N) ----------------
            xcT = npool.tile([P, 2, N], BF16, tag="xcT")
            for nt in range(N // P):
                xw_t = work.tile([P, TFD], BF16, tag="xw_t")
                nc.gpsimd.dma_start(out=xw_t, in_=xw_ext[nt * P:(nt + 1) * P, :])
                for tc2 in range(2):
                    tp = ptp.tile([P, P], BF16, tag="tpbf")
                    nc.tensor.transpose(tp, xw_t[:, tc2 * P:(tc2 + 1) * P],
                                        ident_bf)
                    nc.vector.tensor_copy(out=xcT[:, tc2, nt * P:(nt + 1) * P],
                                          in_=tp)
            xfpT_bf = npool.tile([P, 2], BF16, tag="xfpT_bf")
            nc.vector.tensor_copy(out=xfpT_bf, in_=xfpT)

            # ------------- K/V + attn/denominator accumulation -------------
            def kv_attn_phase(nseq_tiles, recip_dst, make_actT, dump=False,
                              tail_fn=None):
                acc0 = pacc.tile([P, 512], FP32, tag="acc")
                acc1 = pacc.tile([P, 512], FP32, tag="acc")
                acc = [acc0, acc1]
                dT = psmall.tile([P, DC], FP32, tag="small")
                # Interleaved accumulation chains share banks; start=True
                # would clear the whole bank's has_written bits and drop
                # sibling chains' partials. Zero the banks, accumulate with
                # start=False (overwrite-where-clear).
                nc.vector.memset(acc0, 0.0)
                nc.vector.memset(acc1, 0.0)
                nc.vector.memset(dT, 0.0)
                for it in range(nseq_tiles):
                    actT = make_actT(it)
                    ek = work.tile([P, D], BF16, tag="ek_t")
                    vv = work.tile([P, D], BF16, tag="v_t")
                    for w_sb, b_r, is_k in ((Wk_sb, bk_r, True),
                                            (Wv_sb, bv_r, False)):
                        for jh in range(2):
                            ps = proj_to_psum(actT, w_sb, b_r, jh)
                            if is_k:
                                nc.scalar.activation(
                                    out=ek[:, jh * 512:(jh + 1) * 512],
                                    in_=ps, func=AF.Exp)
                            else:
                                nc.scalar.copy(
                                    out=vv[:, jh * 512:(jh + 1) * 512], in_=ps)
                    if dbg and dump:
                        nc.gpsimd.dma_start(out=dext["d_ek"][it], in_=ek)
                        nc.gpsimd.dma_start(out=dext["d_v"][it], in_=vv)
                    last = (it == nseq_tiles - 1) and tail_fn is None
                    for c in range(DC):
                        nc.tensor.matmul(
                            acc[c // 4][:, (c % 4) * P:(c % 4 + 1) * P],
                            lhsT=ek[:, c * P:(c + 1) * P],
                            rhs=vv[:, c * P:(c + 1) * P],
                            start=False, stop=last, skip_group_check=True)
                    for dc in range(DC):
                        nc.tensor.matmul(
                            dT[:, dc:dc + 1],
                            lhsT=ek[:, dc * P:(dc + 1) * P],
                            rhs=onescol_bf,
                            start=False, stop=last, skip_group_check=True)
                if tail_fn is not None:
                    tail_fn(acc, dT)
                nc.vector.reciprocal(out=recip_dst, in_=dT)
                return acc

            # --- n-path (2N = 1024 rows: xw_xf -> tn) ---
            def make_tnT(nt):
                psa = pproj.tile([P, 512], FP32, tag="proj")
                psb = pproj.tile([P, 512], FP32, tag="proj")
                for jh, ps in enumerate((psa, psb)):
                    for tc2 in range(2):
                        if nt < N // P:
                            lhs = xcT[:, tc2, nt * P:(nt + 1) * P]
                        else:
                            lhs = xfpT_bf[:, tc2:tc2 + 1].broadcast_to([P, P])
                        nc.tensor.matmul(
                            ps, lhsT=lhs,
                            rhs=Wat_sb[:, tc2, jh * 512:(jh + 1) * 512],
                            start=(tc2 == 0),
                            stop=(bat_r is None and tc2 == 1))
                    if bat_r is not None:
                        nc.tensor.matmul(ps, lhsT=ones1_bf,
                                         rhs=bat_r[0:1, jh * 512:(jh + 1) * 512],
                                         start=False, stop=True)
                mv = work.tile([P, 2], FP32, tag="mv")
                ln_stats((psa, psb), mv)
                rstd_inplace(mv[:, 1:2])
                tn_t = work.tile([P, D], BF16, tag="tn_t")
                for j, ps in enumerate((psa, psb)):
                    ln_apply(ps, tn_t[:, j * 512:(j + 1) * 512],
                             mv[:, 0:1], mv[:, 1:2], gt_bc, bt_bc,
                             slice(j * 512, (j + 1) * 512))
                tnT = work.tile([P, DC, P], BF16, tag="tnT")
                transpose_to(tn_t, tnT)
                return tnT

            def rep_tail(acc, dT):
                # rows N..2N of the concat are one identical row; its
                # LN/K/V are computed once and folded in scaled by N
                # (exp(K_rep + ln N) = N * exp(K_rep)).
                psa = pproj.tile([1, 512], FP32, tag="proj")
                psb = pproj.tile([1, 512], FP32, tag="proj")
                for jh, ps in enumerate((psa, psb)):
                    for tc2 in range(2):
                        nc.tensor.matmul(
                            ps, lhsT=xfpT_bf[:, tc2:tc2 + 1],
                            rhs=Wat_sb[:, tc2, jh * 512:(jh + 1) * 512],
                            start=(tc2 == 0),
                            stop=(bat_r is None and tc2 == 1))
                    if bat_r is not None:
                        nc.tensor.matmul(
                            ps, lhsT=ones1_bf[0:1, 0:1],
                            rhs=bat_r[0:1, jh * 512:(jh + 1) * 512],
                            start=False, stop=True)
                mvr = work.tile([1, 2], FP32, tag="mvr")
                statsr = work.tile([1, 2, 6], FP32, tag="statsr")
                for j, ps in enumerate((psa, psb)):
                    nc.vector.bn_stats(out=statsr[0:1, j, :], in_=ps)
                nc.vector.bn_aggr(out=mvr, in_=statsr)
                nc.scalar.activation(out=mvr[0:1, 1:2], in_=mvr[0:1, 1:2],
                                     func=AF.Sqrt, bias=eps_t[0:1, :], scale=1.0)
                nc.vector.reciprocal(out=mvr[0:1, 1:2], in_=mvr[0:1, 1:2])
                tn_rep = npool.tile([1, D], BF16, tag="tn_rep")
                for j, ps in enumerate((psa, psb)):
                    if gt_bc is None:
                        nc.vector.tensor_scalar(
                            out=tn_rep[0:1, j * 512:(j + 1) * 512], in0=ps,
                            scalar1=mvr[0:1, 0:1], scalar2=mvr[0:1, 1:2],
                            op0=OP.subtract, op1=OP.mult)
                    else:
                        tmpr = work.tile([1, 512], FP32, tag="tmpr")
                        nc.vector.tensor_scalar(
                            out=tmpr, in0=ps,
                            scalar1=mvr[0:1, 0:1], scalar2=mvr[0:1, 1:2],
                            op0=OP.subtract, op1=OP.mult)
                        nc.vector.tensor_mul(
                            out=tmpr, in0=tmpr,
                            in1=gt_bc[0:1, j * 512:(j + 1) * 512])
                        nc.vector.tensor_add(
                            out=tn_rep[0:1, j * 512:(j + 1) * 512], in0=tmpr,
                            in1=bt_bc[0:1, j * 512:(j + 1) * 512])
                tpr = ptp.tile([P, DC, 2], BF16, tag="tpbf")
                for c in range(DC):
                    nc.tensor.transpose(tpr[:, c, 0:1],
                                        tn_rep[0:1, c * P:(c + 1) * P],
                                        ident_bf[0:1, 0:1])
                tnT_rep = work.tile([P, DC], BF16, tag="tnT_rep")
                nc.vector.tensor_copy(out=tnT_rep[:, :, None],
                                      in_=tpr[:, :, 0:1])
                ekr = npool.tile([1, D], BF16, tag="ekr")
                vrep = npool.tile([1, D], BF16, tag="vrep")
                for w_sb, b_r, is_k in ((Wk_sb, bk_r, True),
                                        (Wv_sb, bv_r, False)):
                    for jh in range(2):
                        ps = pproj.tile([1, 512], FP32, tag="proj")
                        for dc in range(DC):
                            nc.tensor.matmul(
                                ps, lhsT=tnT_rep[:, dc:dc + 1],
                                rhs=w_sb[:, dc, jh * 512:(jh + 1) * 512],
                                start=(dc == 0),
                                stop=(b_r is None and dc == DC - 1))
                        if b_r is not None:
                            nc.tensor.matmul(
                                ps, lhsT=ones1_bf[0:1, 0:1],
                                rhs=b_r[0:1, jh * 512:(jh + 1) * 512],
                                start=False, stop=True)
                        if is_k:
                            nc.scalar.activation(
                                out=ekr[0:1, jh * 512:(jh + 1) * 512],
                                in_=ps, func=AF.Exp, bias=ln512_t[0:1, :])
                        else:
                            nc.scalar.copy(
                                out=vrep[0:1, jh * 512:(jh + 1) * 512], in_=ps)
                for c in range(DC):
                    nc.tensor.matmul(
                        acc[c // 4][:, (c % 4) * P:(c % 4 + 1) * P],
                        lhsT=ekr[0:1, c * P:(c + 1) * P],
                        rhs=vrep[0:1, c * P:(c + 1) * P],
                        start=False, stop=True, skip_group_check=True)
                for dc in range(DC):
                    nc.tensor.matmul(
                        dT[:, dc:dc + 1],
                        lhsT=ekr[0:1, dc * P:(dc + 1) * P],
                        rhs=ones1_bf[0:1, 0:1],
                        start=False, stop=True, skip_group_check=True)

            recipTk = npool.tile([P, DC], FP32, tag="recipTk")
            acc_k = kv_attn_phase(NT if dbg else N // P, recipTk, make_tnT,
                                  dump=dbg, tail_fn=None if dbg else rep_tail)
            if dbg:
                nc.gpsimd.dma_start(out=dext["d_recipk"][:, :], in_=recipTk)

            # scale attn rows by recip_k -> SBUF f32 (drains acc_k)
            attn_sc = npool.tile([P, DC, P], FP32, tag="attn_sc")
            for c in range(DC):
                nc.vector.tensor_scalar_mul(
                    out=attn_sc[:, c, :],
                    in0=acc_k[c // 4][:, (c % 4) * P:(c % 4 + 1) * P],
                    scalar1=recipTk[:, c:c + 1])
            if dbg:
                nc.gpsimd.dma_start(out=dext["d_attnsc"][:, :, :], in_=attn_sc)

            # --- s-path (S = 512 rows: xs -> sn) ---
            def make_snT(st):
                xs_t = work.tile([P, D], FP32, tag="xs_t")
                nc.sync.dma_start(out=xs_t, in_=xs_ext[st * P:(st + 1) * P, :])
                mv = work.tile([P, 2], FP32, tag="mv")
                ln_stats((xs_t[:, 0:512], xs_t[:, 512:1024]), mv)
                rstd_inplace(mv[:, 1:2])
                sn_t = work.tile([P, D], BF16, tag="tn_t")
                if gs_bc is None:
                    nc.vector.tensor_scalar(
                        out=sn_t, in0=xs_t, scalar1=mv[:, 0:1],
                        scalar2=mv[:, 1:2], op0=OP.subtract, op1=OP.mult)
                else:
                    for j in range(2):
                        ln_apply(xs_t[:, j * 512:(j + 1) * 512],
                                 sn_t[:, j * 512:(j + 1) * 512],
                                 mv[:, 0:1], mv[:, 1:2], gs_bc, bs_bc,
                                 slice(j * 512, (j + 1) * 512))
                snT = work.tile([P, DC, P], BF16, tag="tnT")
                transpose_to(sn_t, snT)
                return snT

            recipTs = npool.tile([P, DC], FP32, tag="recipTs")
            acc_s = kv_attn_phase(ST, recipTs, make_snT)
            if dbg:
                nc.gpsimd.dma_start(out=dext["d_recips"][:, :], in_=recipTs)

            # sattnsum rows: scale G rows by recip_s, mask cross-head
            # terms, column-sum (col l then sums only d in head(l)).
            sattn_row = npool.tile([1, DC, P], FP32, tag="sattn_row")
            for c in range(DC):
                gsc = work.tile([P, P], FP32, tag="gsc")
                nc.vector.tensor_scalar_mul(
                    out=gsc, in0=acc_s[c // 4][:, (c % 4) * P:(c % 4 + 1) * P],
                    scalar1=recipTs[:, c:c + 1])
                nc.vector.tensor_mul(out=gsc, in0=gsc, in1=mask_f)
                ssp = psmall.tile([1, P], FP32, tag="small")
                nc.tensor.matmul(ssp, lhsT=onescol_f, rhs=gsc,
                                 start=True, stop=True)
                nc.vector.tensor_copy(out=sattn_row[0:1, c, :], in_=ssp)
            if dbg:
                nc.gpsimd.dma_start(out=dext["d_sattn"][:, :, :], in_=sattn_row)

            # ---------------- attn2 block-diagonal tiles ----------------
            attn2 = npool.tile([P, DC, P], BF16, tag="attn2")
            for c in range(DC):
                psb = psmall.tile([P, P], FP32, tag="small")
                nc.tensor.matmul(psb, lhsT=ones1_f, rhs=sattn_row[0:1, c, :],
                                 start=True, stop=True)
                tmp = work.tile([P, P], FP32, tag="a2tmp")
                nc.vector.tensor_add(out=tmp, in0=attn_sc[:, c, :], in1=psb)
                nc.vector.tensor_mul(out=attn2[:, c, :], in0=tmp, in1=mask_f)
            if dbg:
                nc.gpsimd.dma_start(out=dext["d_attn2"][:, :, :], in_=attn2)

        # ---------------- x-path: 32 token tiles, paired ----------------
        with tc.tile_pool(name="pxtra", bufs=2, space="PSUM") as pxtra:

            def stage1(tt):
                """load, LN, transpose, q-proj, exp -> (x_sb, Eq)"""
                x_sb = xin.tile([P, D], FP32, tag="x_sb")
                nc.sync.dma_start(out=x_sb, in_=x_ext[tt * P:(tt + 1) * P, :])
                xln = chain.tile([P, D], BF16, tag="xln")
                if gx_bc is None:
                    nc.vector.tensor_scalar(
                        out=xln, in0=x_sb, scalar1=mvall[:, tt, 0:1],
                        scalar2=mvall[:, tt, 1:2], op0=OP.subtract, op1=OP.mult)
                else:
                    for j in range(2):
                        ln_apply(x_sb[:, j * 512:(j + 1) * 512],
                                 xln[:, j * 512:(j + 1) * 512],
                                 mvall[:, tt, 0:1], mvall[:, tt, 1:2],
                                 gx_bc, bx_bc, slice(j * 512, (j + 1) * 512))
                xlT = chain.tile([P, DC, P], BF16, tag="xlT")
                transpose_to(xln, xlT, copy_eng="scalar")
                Eq = chain.tile([P, D], BF16, tag="Eq")
                for jh in range(2):
                    ps = proj_to_psum(xlT, Wq_sb, bq_r, jh)
                    nc.scalar.activation(out=Eq[:, jh * 512:(jh + 1) * 512],
                                         in_=ps, func=AF.Exp)
                return x_sb, Eq

            def stage2(tt, Eq):
                """softmax over dh-groups -> qsmT"""
                dsum = work.tile([P, H], FP32, tag="dsum")
                nc.vector.reduce_sum(out=dsum,
                                     in_=Eq.rearrange("p (h d) -> p h d", h=H),
                                     axis=AX)
                nc.vector.reciprocal(out=dsum, in_=dsum)
                dsum_bf = work.tile([P, H], BF16, tag="dsum_bf")
                nc.vector.tensor_copy(out=dsum_bf, in_=dsum)
                qsm = chain.tile([P, D], BF16, tag="qsm")
                nc.vector.tensor_mul(
                    out=qsm.rearrange("p (h d) -> p h d", h=H),
                    in0=Eq.rearrange("p (h d) -> p h d", h=H),
                    in1=dsum_bf[:, :, None].broadcast_to([P, H, dh]))
                if dbg:
                    nc.gpsimd.dma_start(out=dext["d_eq"][tt], in_=Eq)
                    nc.gpsimd.dma_start(out=dext["d_qsm"][tt], in_=qsm)
                qsmT = chain.tile([P, DC, P], BF16, tag="qsmT")
                transpose_to(qsm, qsmT, copy_eng="scalar")
                return qsmT

            def stage3(tt, qsmT):
                """y^T blocks + silu -> siluT"""
                siluT = chain.tile([P, DC, P], BF16, tag="siluT")
                for g in range(0, DC, 4):
                    yps = pxtra.tile([P, 512], FP32, tag="ypsum")
                    for k in range(4):
                        c = g + k
                        nc.tensor.matmul(yps[:, k * P:(k + 1) * P],
                                         lhsT=attn2[:, c, :], rhs=qsmT[:, c, :],
                                         start=True, stop=True)
                    nc.scalar.activation(
                        out=siluT[:, g:g + 4, :].rearrange("p a b -> p (a b)"),
                        in_=yps, func=AF.Silu)
                if dbg:
                    nc.gpsimd.dma_start(out=dext["d_siluT"][tt], in_=siluT)
                return siluT

            def stage4(tt, siluT, x_sb):
                """out-proj + residual + store"""
                o_sb = chain.tile([P, D], FP32, tag="o_sb")
                for jh in range(2):
                    ps = pxtra.tile([P, 512], FP32, tag="opsum")
                    for c in range(DC):
                        nc.tensor.matmul(
                            ps, lhsT=siluT[:, c, :],
                            rhs=Wo_sb[:, c, jh * 512:(jh + 1) * 512],
                            start=(c == 0),
                            stop=(bo_r is None and c == DC - 1))
                    if bo_r is not None:
                        nc.tensor.matmul(ps, lhsT=ones1_bf,
                                         rhs=bo_r[0:1, jh * 512:(jh + 1) * 512],
                                         start=False, stop=True)
                    nc.vector.tensor_add(
                        out=o_sb[:, jh * 512:(jh + 1) * 512], in0=ps,
                        in1=x_sb[:, jh * 512:(jh + 1) * 512])
                nc.sync.dma_start(out=out_ext[tt * P:(tt + 1) * P, :], in_=o_sb)

            for pair in range(TT // 2):
                t0, t1 = 2 * pair, 2 * pair + 1
                xa, Ea = stage1(t0)
                xb, Eb = stage1(t1)
                qa = stage2(t0, Ea)
                qb = stage2(t1, Eb)
                sa = stage3(t0, qa)
                sb = stage3(t1, qb)
                stage4(t0, sa, xa)
                stage4(t1, sb, xb)

    nc.compile()
    return nc


def kernel(**inputs) -> np.ndarray:
    from concourse.bass_utils import run_bass_kernel_spmd

    ins = {k: np.ascontiguousarray(np.asarray(v, dtype=np.float32))
           for k, v in inputs.items()}
    affine_x = not (np.all(ins["norm_g"] == 1.0) and np.all(ins["norm_b"] == 0.0))
    affine_t = not (np.all(ins["tnorm_g"] == 1.0) and np.all(ins["tnorm_b"] == 0.0))
    affine_s = not (np.all(ins["snorm_g"] == 1.0) and np.all(ins["snorm_b"] == 0.0))
    hasb = {nm: bool(np.any(ins[nm] != 0.0))
            for nm in ("bq", "bk", "bv", "ba", "bat", "bo")}

    key = (affine_x, affine_t, affine_s, tuple(sorted(hasb.items())))
    if key not in _CACHE:
        _CACHE[key] = _build(affine_x, affine_t, affine_s, hasb)
    nc = _CACHE[key]

    wnames = ["norm_g", "norm_b", "tnorm_g", "tnorm_b", "snorm_g", "snorm_b",
              "Wq", "bq", "Wk", "bk", "Wv", "bv", "Wa", "ba", "Wat", "bat",
              "Wo", "bo"]
    in_maps = []
    for b in range(NCORES):
        m = {"x": ins["x"][b], "xf": ins["xf"][b], "xw": ins["xw"][b],
             "xs": ins["xs"][b]}
        for nm in wnames:
            m[nm] = ins[nm]
        in_maps.append(m)

    res = run_bass_kernel_spmd(nc, in_maps, core_ids=list(range(NCORES)))
    return np.stack([res.results[i]["out"] for i in range(NCORES)], axis=0)


if __name__ == "__main__":
    import reference
    rin = reference.setup_inputs()
    out = kernel(**{k: np.asarray(v) for k, v in rin.items()})
    print("out shape:", out.shape, out.dtype)
